# revision 12
# baseline (speedup 1.0000x reference)
"""MultiHeadAttention kernel for Trainium2, 8 NeuronCores.

Problem shapes (hardcoded): B=4, C=256, N=M=4096, H=4 heads, D=64 head dim.
reference: Q/K/V = 1x1-conv projections, scores = Q^T K / sqrt(D) per head,
softmax over source dim, out = attn @ V^T, merge projection.

Sharding: 8 cores = (batch b, query-half nh). Each core computes the full
output rows for its (b, n-half): K/V projections are recomputed per n-half
(5% redundant FLOPs) which keeps every core's output slice disjoint — the
host just concatenates, no reductions.

Per-core dataflow (bf16 matmul operands, fp32 PSUM accumulation):
  K  = WkT.T @ src            (c_out on partitions, m free)    [PE]
  Q  = WqT.T @ q              (c_out on partitions, n free)    [PE]
  VT = src.T @ WvT            (m on partitions, c_out free)    [PE]
  attention runs on HEAD PAIRS (2ch, 2ch+1): their score matmuls contract
  only D=64, so the pair occupies PE row-groups 0-63 / 64-127 (tile_position
  auto-derived from base_partition) and the two matmuls execute
  CONCURRENTLY in the systolic array — scores PE time ~halves.
  per (pair ch, n-tile of 1024, m-chunk of 128):
    scoresT_h[m,n] = K_h^T Q_h  both heads, row-tiled          [PE]
    probs = exp(scores/8): one head's chunk on ACT (LUT exp), the
    other on DVE via a bf16 Schraudolph bit-trick:
      bits16 = round(A*s + B), A = 128*log2(e)/8, viewed as bf16
    alternating per m-chunk so every softmax row mixes both paths
    (rel err ~7e-3 end to end)                                 [ACT+DVE]
    outT_h(65,1024) += probs_chunk.T @ [VT_h | ones]  -> row 64 is
    the softmax denominator                                    [PE]
  pair drain: denominators batched (2,1024) -> approx-NR recip [DVE],
    partition-broadcast via a DRAM bounce [DMA], attn_h = outT * r [DVE]
  y = WmT.T @ attn + bm       contract heads, K=64 each        [PE]

Engine balance (the previous version was ACT(exp)-bound at ~343us busy):
exp is split ~50/50 ACT/DVE; K/Q/merge PSUM->SBUF bias-copies run on ACT
(activation Copy with per-partition bias AP); VT bias-adds + attn
normalization stay on DVE. PE does ~200us, ACT/DVE ~200us each.

Hardware landmines (kept working around, see git history of this file):
  - gpsimd.partition_broadcast reads the wrong partition for inputs not
    based at partition 0, and heavy gpsimd SBUF traffic locks the
    DVE-shared port;
  - DMA with partition-shifted or partition-step-0 SBUF APs hangs the
    device (DRAM-side broadcast APs are fine);
  - DVE ops are partition-locked (out/in must share the partition base),
    though a plain reciprocal/copy CAN shift base; the custom-DVE
    reciprocal_approx ops cannot;
  - matmul out must stay within one PSUM bank (<=512 fp32 free).
"""

import os

import numpy as np

N_CORES = 8
B, C = 4, 256
N = M = 4096
H, D = 4, 64
NN = N // 2          # query positions per core
P = 128
NT = NN // 1024      # n-tiles per core (2)
MC = M // P          # m-chunks (32)
LAG = 4              # outT matmuls trail scores by LAG m-chunks

# Schraudolph bf16 exp: bits16 = round(SCHR_A * s + SCHR_B); includes the
# 1/sqrt(D)=0.125 score scaling. B centers the log-error (C_adj ~ 7.5).
SCHR_A = float(128.0 * 0.125 / np.log(2.0))
SCHR_B = float(128.0 * 127.0 - 7.5)

_STATE: dict = {}


def _build():
    from contextlib import ExitStack

    import concourse.bass as bass
    import concourse.mybir as mybir
    import concourse.tile as tile
    from concourse import bacc

    f32 = mybir.dt.float32
    bf16 = mybir.dt.bfloat16
    i16 = mybir.dt.int16
    Exp = mybir.ActivationFunctionType.Exp
    Ident = mybir.ActivationFunctionType.Identity
    add = mybir.AluOpType.add
    mult = mybir.AluOpType.mult

    nc = bacc.Bacc(
        "TRN2",
        target_bir_lowering=False,
        debug=False,
        enable_asserts=False,
        num_devices=N_CORES,
    )

    q_d = nc.dram_tensor("q", (C, NN), bf16, kind="ExternalInput").ap()
    src_d = nc.dram_tensor("src", (C, M), bf16, kind="ExternalInput").ap()
    wqT_d = nc.dram_tensor("wqT", (C, C), bf16, kind="ExternalInput").ap()
    wkT_d = nc.dram_tensor("wkT", (C, C), bf16, kind="ExternalInput").ap()
    wvT_d = nc.dram_tensor("wvT", (C, C), bf16, kind="ExternalInput").ap()
    wmT_d = nc.dram_tensor("wmT", (C, C), bf16, kind="ExternalInput").ap()
    bq_d = nc.dram_tensor("bq", (C,), f32, kind="ExternalInput").ap()
    bk_d = nc.dram_tensor("bk", (C,), f32, kind="ExternalInput").ap()
    bv_d = nc.dram_tensor("bv", (C,), f32, kind="ExternalInput").ap()
    bm_d = nc.dram_tensor("bm", (C,), f32, kind="ExternalInput").ap()
    y_d = nc.dram_tensor("y", (C, NN), f32, kind="ExternalOutput").ap()

    q_r = q_d.rearrange("(a p) n -> a p n", p=P)
    src_r = src_d.rearrange("(a p) n -> a p n", p=P)
    y_r = y_d.rearrange("(a p) n -> a p n", p=P)

    def chunks(w):
        return w.rearrange("(a p) n -> a p n", p=P)

    with tile.TileContext(nc) as tc, ExitStack() as ctx:
        singles = ctx.enter_context(tc.tile_pool(name="singles", bufs=1))
        # PSUM: scores 2 tiles x 2 banks + outT (two live head
        # accumulators) 2 x 2 banks = 8 banks. Projections/VT borrow the
        # outT slots (they run strictly outside the attention pairs).
        spool = ctx.enter_context(tc.tile_pool(name="scores", bufs=2, space="PSUM"))
        opool = ctx.enter_context(tc.tile_pool(name="outps", bufs=2, space="PSUM"))
        probs_p = ctx.enter_context(tc.tile_pool(name="probs", bufs=2 * (LAG + 1)))
        small_p = ctx.enter_context(tc.tile_pool(name="small", bufs=2))
        dram_p = ctx.enter_context(tc.tile_pool(name="dram", bufs=2, space="DRAM"))
        ostage = ctx.enter_context(tc.tile_pool(name="ostage", bufs=3))

        # ---- weights / biases ----
        wqt, wkt, wvt = [], [], []
        for ci in range(2):
            for lst, d, nm in ((wqt, wqT_d, "wq"), (wkt, wkT_d, "wk"),
                               (wvt, wvT_d, "wv")):
                t = singles.tile([P, C], bf16, tag=f"{nm}{ci}", name=f"{nm}{ci}")
                nc.gpsimd.dma_start(out=t[:], in_=chunks(d)[ci])
                lst.append(t)
        wm_h = []
        for h in range(H):
            t = singles.tile([D, C], bf16, tag=f"wm{h}", name=f"wm{h}")
            nc.gpsimd.dma_start(out=t[:], in_=wmT_d[h * D:(h + 1) * D, :])
            wm_h.append(t)
        bq_t, bk_t, bm_t = [], [], []
        for ci in range(2):
            for lst, d, nm in ((bq_t, bq_d, "bq"), (bk_t, bk_d, "bk"),
                               (bm_t, bm_d, "bm")):
                t = singles.tile([P, 1], f32, tag=f"{nm}{ci}", name=f"b{nm}{ci}")
                nc.gpsimd.dma_start(out=t[:], in_=d.rearrange("(a p) -> a p", p=P)[ci][:, None])
                lst.append(t)
        bv_rep = singles.tile([P, C], f32, tag="bv_rep", name="bv_rep")
        nc.gpsimd.dma_start(
            out=bv_rep[:],
            in_=bass.AP(tensor=bv_d.tensor, offset=bv_d.offset,
                        ap=[[0, P]] + list(bv_d.ap)),
        )

        # ---- persistent activations ----
        Q_sb = [singles.tile([P, NN], bf16, tag=f"Q{ci}", name=f"Q{ci}") for ci in range(2)]
        K_sb = [singles.tile([P, M], bf16, tag=f"K{ci}", name=f"K{ci}") for ci in range(2)]
        vt = singles.tile([P, MC, H, D + 1], bf16, tag="vt", name="vt")
        attn = [singles.tile([D, NN], bf16, tag=f"attn{h}", name=f"attn{h}") for h in range(H)]

        nc.vector.memset(vt[:, :, :, D:D + 1], 1.0)

        # ---- inputs (chunked DMAs so the first matmul starts early) ----
        inp = ctx.enter_context(tc.tile_pool(name="inp", bufs=1))
        src_t = [inp.tile([P, M], bf16, tag=f"srcin{ci}", name=f"srcin{ci}")
                 for ci in range(2)]
        q_t = [inp.tile([P, NN], bf16, tag=f"qin{ci}", name=f"qin{ci}")
               for ci in range(2)]
        for c4 in range(8):       # src first (K then VT proj need it first),
            for ci in range(2):   # column-chunked so t=0 lands quickly,
                eng = nc.sync if ci == 0 else nc.scalar  # two DGE queues
                eng.dma_start(out=src_t[ci][:, c4 * 512:(c4 + 1) * 512],
                              in_=src_r[ci][:, c4 * 512:(c4 + 1) * 512])
        for c4 in range(4):
            for ci in range(2):
                nc.gpsimd.dma_start(out=q_t[ci][:, c4 * 512:(c4 + 1) * 512],
                                    in_=q_r[ci][:, c4 * 512:(c4 + 1) * 512])

        # ---- projections (bf16 inputs, fp32 PSUM accumulate) ----
        def proj_one(co, wt, xin, xlen, dst, bias):
            # c_out partitions, sequence on free dim; the PSUM->SBUF copy
            # with per-partition bias runs on ACT (idle outside exp here).
            # PSUM comes from the scores pool so these matmuls never wait
            # on the attention drain holding the outT accumulators.
            for t in range(xlen // 512):
                ps = spool.tile([P, 1024], f32, tag="sc", name="ps")[:, 0:512]
                for ci in range(2):
                    nc.tensor.matmul(
                        ps[:],
                        wt[ci][:, co * P:(co + 1) * P],
                        xin[ci][:, t * 512:(t + 1) * 512],
                        start=(ci == 0), stop=(ci == 1),
                    )
                nc.scalar.activation(
                    out=dst[co][:, t * 512:(t + 1) * 512], in_=ps[:],
                    func=Ident, bias=bias[co][:])

        def proj_vt_chunk(mc):
            # VT = src.T @ WvT (m partitions, c_out free) + bv, stored as
            # per-head [VT_h | ones] blocks of width D+1
            ps = spool.tile([P, 1024], f32, tag="sc", name="psv")[:, 0:C]
            for ci in range(2):
                nc.tensor.matmul(
                    ps[:],
                    src_t[ci][:, mc * P:(mc + 1) * P],
                    wvt[ci][:],
                    start=(ci == 0), stop=(ci == 1),
                )
            nc.vector.tensor_tensor(
                vt[:, mc, :, 0:D],
                ps.rearrange("p (h d) -> p h d", h=H),
                bv_rep.rearrange("p (h d) -> p h d", h=H),
                add,
            )

        # ---- attention on head pairs ----
        def attention_pair(ch, nt):
            h0, h1 = 2 * ch, 2 * ch + 1
            n0 = nt * 1024
            outT = [opool.tile([D + 1, 1024], f32, tag="outT", name=f"oT{w}")
                    for w in range(2)]
            prs = {}
            for mc in range(MC + LAG):
                if mc < MC:
                    sc = [spool.tile([P, 1024], f32, tag="sc", name=f"sc{w}")
                          for w in range(2)]
                    # interleave the two heads' score matmuls: head w sits
                    # at PE row-group w*64 (auto tile_position), so the
                    # pair runs concurrently in the array
                    for ns in range(2):
                        for w, off in ((0, 0), (1, D)):
                            nc.tensor.matmul(
                                sc[w][:, ns * 512:(ns + 1) * 512],
                                K_sb[ch][off:off + D, mc * P:(mc + 1) * P],
                                Q_sb[ch][off:off + D,
                                         n0 + ns * 512:n0 + (ns + 1) * 512],
                                start=True, stop=True,
                            )
                    for w in range(2):
                        pr = probs_p.tile([P, 1024], bf16, tag="pr", name=f"pr{w}")
                        if (mc + w) % 2 == 0:
                            nc.vector.tensor_scalar(
                                pr[:].bitcast(i16), sc[w][:],
                                SCHR_A, SCHR_B, mult, add)
                        else:
                            nc.scalar.activation(
                                out=pr[:], in_=sc[w][:], func=Exp, scale=0.125)
                        prs[(mc, w)] = pr
                if mc >= LAG:
                    j = mc - LAG
                    for w, h in ((0, h0), (1, h1)):
                        pr_j = prs.pop((j, w))
                        for ns in range(2):
                            nc.tensor.matmul(
                                outT[w][:, ns * 512:(ns + 1) * 512],
                                vt[:, j, h, :],
                                pr_j[:, ns * 512:(ns + 1) * 512],
                                start=(j == 0), stop=(j == MC - 1),
                            )
            # drain: copy raw outT (incl. denominator row D) to SBUF right
            # away — one head on ACT, one on DVE — so the PSUM accumulators
            # free within the LAG window and the PE never stalls (a stalled
            # PE re-throttles the HAM clock 2.4->1.2 GHz). The recip/
            # broadcast/normalize chain then runs off the critical path.
            uout = [small_p.tile([D + 1, 1024], f32, tag=f"uo{w}", name=f"uo{w}")
                    for w in range(2)]
            nc.scalar.copy(out=uout[0][:], in_=outT[0][:])
            nc.vector.tensor_copy(out=uout[1][:], in_=outT[1][:])
            den = small_p.tile([1, 2048], f32, tag="den", name="den")
            for w in range(2):
                nc.vector.tensor_copy(out=den[0:1, w * 1024:(w + 1) * 1024],
                                      in_=uout[w][D:D + 1, :])
            rec = small_p.tile([1, 2048], f32, tag="rec", name="rec")
            scr = small_p.tile([1, 2048], f32, tag="scr", name="scr")
            nc.vector.reciprocal_approx_accurate(
                out=rec[0:1, :], in_=den[0:1, :], scratch=scr[0:1, :])
            dscr = dram_p.tile([1, 2048], f32, name="dscr")
            nc.sync.dma_start(out=dscr[:], in_=rec[0:1, :])
            for w, h in ((0, h0), (1, h1)):
                row = dscr[0:1, w * 1024:(w + 1) * 1024]
                rrep = small_p.tile([D, 1024], f32, tag=f"rrep{w}", name=f"rrep{w}")
                nc.sync.dma_start(
                    out=rrep[:],
                    in_=bass.AP(tensor=row.tensor, offset=row.offset,
                                ap=[[0, D]] + list(row.ap)[1:]))
                nc.vector.tensor_tensor(
                    attn[h][:, n0:n0 + 1024],
                    uout[w][0:D, :],
                    rrep[:],
                    mult,
                )

        def merge_nt(nt):
            for co in range(2):
                for t in range(2 * nt, 2 * nt + 2):
                    ps = spool.tile([P, 1024], f32, tag="sc", name="psm")[:, 0:512]
                    for h in range(H):
                        nc.tensor.matmul(
                            ps[:],
                            wm_h[h][:, co * P:(co + 1) * P],
                            attn[h][:, t * 512:(t + 1) * 512],
                            start=(h == 0), stop=(h == H - 1),
                        )
                    ot = ostage.tile([P, 512], f32, tag="ot", name="ot")
                    nc.scalar.activation(out=ot[:], in_=ps[:],
                                         func=Ident, bias=bm_t[co][:])
                    nc.sync.dma_start(out=y_r[co, :, t * 512:(t + 1) * 512],
                                      in_=ot[:])

        proj_one(0, wkt, src_t, M, K_sb, bk_t)
        proj_one(0, wqt, q_t, NN, Q_sb, bq_t)
        for mc in range(MC):
            proj_vt_chunk(mc)
        attention_pair(0, 0)
        attention_pair(0, 1)
        proj_one(1, wkt, src_t, M, K_sb, bk_t)
        proj_one(1, wqt, q_t, NN, Q_sb, bq_t)
        attention_pair(1, 0)
        attention_pair(1, 1)
        merge_nt(0)
        merge_nt(1)

    nc.compile()
    return nc


def _get_nc():
    if "nc" not in _STATE:
        _STATE["nc"] = _build()
    return _STATE["nc"]


def kernel(query, source, Wq, bq, Wk, bk, Wv, bv, Wm, bm):
    import ml_dtypes
    from concourse.bass_utils import run_bass_kernel_spmd

    bf16 = ml_dtypes.bfloat16
    query = np.asarray(query, np.float32).astype(bf16)
    source = np.asarray(source, np.float32).astype(bf16)
    wqT = np.ascontiguousarray(np.asarray(Wq, np.float32).T).astype(bf16)
    wkT = np.ascontiguousarray(np.asarray(Wk, np.float32).T).astype(bf16)
    wvT = np.ascontiguousarray(np.asarray(Wv, np.float32).T).astype(bf16)
    wmT = np.ascontiguousarray(np.asarray(Wm, np.float32).T).astype(bf16)
    bq = np.asarray(bq, np.float32)
    bk = np.asarray(bk, np.float32)
    bv = np.asarray(bv, np.float32)
    bm = np.asarray(bm, np.float32)

    nc = _get_nc()

    in_maps = []
    for c in range(N_CORES):
        b, nh = c // 2, c % 2
        in_maps.append({
            "q": np.ascontiguousarray(query[b, :, nh * NN:(nh + 1) * NN]),
            "src": np.ascontiguousarray(source[b]),
            "wqT": wqT, "wkT": wkT, "wvT": wvT, "wmT": wmT,
            "bq": bq, "bk": bk, "bv": bv, "bm": bm,
        })

    trace = os.environ.get("KERNEL_TRACE") == "1"
    res = run_bass_kernel_spmd(
        nc, in_maps, core_ids=list(range(N_CORES)), trace=trace)
    _STATE["last_result"] = res
    if trace and res.exec_time_ns is not None:
        print(f"HW exec time: {res.exec_time_ns} ns")

    out = np.empty((B, C, N), np.float32)
    for c in range(N_CORES):
        b, nh = c // 2, c % 2
        out[b, :, nh * NN:(nh + 1) * NN] = res.results[c]["y"]
    return out


# revision 16
# speedup vs baseline: 1.2156x; 1.2156x over previous
"""MultiHeadAttention kernel for Trainium2, 8 NeuronCores.

Problem shapes (hardcoded): B=4, C=256, N=M=4096, H=4 heads, D=64 head dim.
reference: Q/K/V = 1x1-conv projections, scores = Q^T K / sqrt(D) per head,
softmax over source dim, out = attn @ V^T, merge projection.

Sharding: 8 cores = (batch b, query-half nh). Each core computes the full
output rows for its (b, n-half): K/V projections are recomputed per n-half
(5% redundant FLOPs) which keeps every core's output slice disjoint — the
host just concatenates, no reductions.

Per-core dataflow (bf16 matmul operands, fp32 PSUM accumulation):
  K  = WkT.T @ src            (c_out on partitions, m free)    [PE]
  Q  = WqT.T @ q              (c_out on partitions, n free)    [PE]
  VT = src.T @ WvT            (m on partitions, c_out free)    [PE]
  attention runs on HEAD PAIRS (2ch, 2ch+1): their score matmuls contract
  only D=64, so the pair occupies PE row-groups 0-63 / 64-127 (tile_position
  auto-derived from base_partition) and the two matmuls execute
  CONCURRENTLY in the systolic array — scores PE time ~halves.
  per (pair ch, n-tile of 1024, m-chunk of 128):
    scoresT_h[m,n] = K_h^T Q_h  both heads, row-tiled          [PE]
    probs = exp(scores/8): one head's chunk on ACT (LUT exp), the
    other on DVE via a bf16 Schraudolph bit-trick:
      bits16 = round(A*s + B), A = 128*log2(e)/8, viewed as bf16
    alternating per m-chunk so every softmax row mixes both paths
    (rel err ~7e-3 end to end)                                 [ACT+DVE]
    outT_h(65,1024) += probs_chunk.T @ [VT_h | ones]  -> row 64 is
    the softmax denominator                                    [PE]
  pair drain: denominators batched (2,1024) -> approx-NR recip [DVE],
    partition-broadcast via a DRAM bounce [DMA], attn_h = outT * r [DVE]
  y = WmT.T @ attn + bm       contract heads, K=64 each        [PE]

Engine balance (the previous version was ACT(exp)-bound at ~343us busy):
exp is split ~50/50 ACT/DVE; K/Q/merge PSUM->SBUF bias-copies run on ACT
(activation Copy with per-partition bias AP); VT bias-adds + attn
normalization stay on DVE. PE does ~200us, ACT/DVE ~200us each.

Hardware landmines (kept working around, see git history of this file):
  - gpsimd.partition_broadcast reads the wrong partition for inputs not
    based at partition 0, and heavy gpsimd SBUF traffic locks the
    DVE-shared port;
  - DMA with partition-shifted or partition-step-0 SBUF APs hangs the
    device (DRAM-side broadcast APs are fine);
  - DVE ops are partition-locked (out/in must share the partition base),
    though a plain reciprocal/copy CAN shift base; the custom-DVE
    reciprocal_approx ops cannot;
  - matmul out must stay within one PSUM bank (<=512 fp32 free).
"""

import os

import numpy as np

N_CORES = 8
B, C = 4, 256
N = M = 4096
H, D = 4, 64
NN = N // 2          # query positions per core
P = 128
NT = NN // 1024      # n-tiles per core (2)
MC = M // P          # m-chunks (32)
LAG = 4              # outT matmuls trail scores by LAG m-chunks

# Schraudolph bf16 exp: bits16 = round(SCHR_A * s + SCHR_B); includes the
# 1/sqrt(D)=0.125 score scaling. B centers the log-error (C_adj ~ 7.5).
SCHR_A = float(128.0 * 0.125 / np.log(2.0))
SCHR_B = float(128.0 * 127.0 - 7.5)

_STATE: dict = {}


def _build():
    from contextlib import ExitStack

    import concourse.bass as bass
    import concourse.mybir as mybir
    import concourse.tile as tile
    from concourse import bacc

    f32 = mybir.dt.float32
    bf16 = mybir.dt.bfloat16
    i16 = mybir.dt.int16
    Exp = mybir.ActivationFunctionType.Exp
    Ident = mybir.ActivationFunctionType.Identity
    add = mybir.AluOpType.add
    mult = mybir.AluOpType.mult

    nc = bacc.Bacc(
        "TRN2",
        target_bir_lowering=False,
        debug=False,
        enable_asserts=False,
        num_devices=N_CORES,
    )

    q_d = nc.dram_tensor("q", (C, NN), bf16, kind="ExternalInput").ap()
    src_d = nc.dram_tensor("src", (C, M), bf16, kind="ExternalInput").ap()
    wqT_d = nc.dram_tensor("wqT", (C, C), bf16, kind="ExternalInput").ap()
    wkT_d = nc.dram_tensor("wkT", (C, C), bf16, kind="ExternalInput").ap()
    wvT_d = nc.dram_tensor("wvT", (C, C), bf16, kind="ExternalInput").ap()
    wmT_d = nc.dram_tensor("wmT", (C, C), bf16, kind="ExternalInput").ap()
    bq_d = nc.dram_tensor("bq", (C,), f32, kind="ExternalInput").ap()
    bk_d = nc.dram_tensor("bk", (C,), f32, kind="ExternalInput").ap()
    bv_d = nc.dram_tensor("bv", (C,), f32, kind="ExternalInput").ap()
    bm_d = nc.dram_tensor("bm", (C,), f32, kind="ExternalInput").ap()
    y_d = nc.dram_tensor("y", (C, NN), f32, kind="ExternalOutput").ap()

    q_r = q_d.rearrange("(a p) n -> a p n", p=P)
    src_r = src_d.rearrange("(a p) n -> a p n", p=P)
    y_r = y_d.rearrange("(a p) n -> a p n", p=P)

    def chunks(w):
        return w.rearrange("(a p) n -> a p n", p=P)

    with tile.TileContext(nc) as tc, ExitStack() as ctx:
        singles = ctx.enter_context(tc.tile_pool(name="singles", bufs=1))
        # PSUM: scores 2 tiles x 2 banks + outT (two live head
        # accumulators) 2 x 2 banks = 8 banks. Projections/VT borrow the
        # outT slots (they run strictly outside the attention pairs).
        spool = ctx.enter_context(tc.tile_pool(name="scores", bufs=2, space="PSUM"))
        opool = ctx.enter_context(tc.tile_pool(name="outps", bufs=2, space="PSUM"))
        probs_p = ctx.enter_context(tc.tile_pool(name="probs", bufs=2 * (LAG + 1)))
        small_p = ctx.enter_context(tc.tile_pool(name="small", bufs=2))
        dram_p = ctx.enter_context(tc.tile_pool(name="dram", bufs=2, space="DRAM"))
        ostage = ctx.enter_context(tc.tile_pool(name="ostage", bufs=3))

        # ---- weights / biases ----
        wqt, wkt, wvt = [], [], []
        for ci in range(2):
            for lst, d, nm in ((wqt, wqT_d, "wq"), (wkt, wkT_d, "wk"),
                               (wvt, wvT_d, "wv")):
                t = singles.tile([P, C], bf16, tag=f"{nm}{ci}", name=f"{nm}{ci}")
                nc.gpsimd.dma_start(out=t[:], in_=chunks(d)[ci])
                lst.append(t)
        wm_h = []
        for h in range(H):
            t = singles.tile([D, C], bf16, tag=f"wm{h}", name=f"wm{h}")
            nc.gpsimd.dma_start(out=t[:], in_=wmT_d[h * D:(h + 1) * D, :])
            wm_h.append(t)
        bq_t, bk_t, bm_t = [], [], []
        for ci in range(2):
            for lst, d, nm in ((bq_t, bq_d, "bq"), (bk_t, bk_d, "bk"),
                               (bm_t, bm_d, "bm")):
                t = singles.tile([P, 1], f32, tag=f"{nm}{ci}", name=f"b{nm}{ci}")
                nc.gpsimd.dma_start(out=t[:], in_=d.rearrange("(a p) -> a p", p=P)[ci][:, None])
                lst.append(t)
        bv_rep = singles.tile([P, C], f32, tag="bv_rep", name="bv_rep")
        nc.gpsimd.dma_start(
            out=bv_rep[:],
            in_=bass.AP(tensor=bv_d.tensor, offset=bv_d.offset,
                        ap=[[0, P]] + list(bv_d.ap)),
        )

        # ---- persistent activations ----
        Q_sb = [singles.tile([P, NN], bf16, tag=f"Q{ci}", name=f"Q{ci}") for ci in range(2)]
        K_sb = [singles.tile([P, M], bf16, tag=f"K{ci}", name=f"K{ci}") for ci in range(2)]
        vt = singles.tile([P, MC, H, D + 1], bf16, tag="vt", name="vt")
        attn = [singles.tile([D, NN], bf16, tag=f"attn{h}", name=f"attn{h}") for h in range(H)]

        nc.vector.memset(vt[:, :, :, D:D + 1], 1.0)

        # ---- inputs (chunked DMAs so the first matmul starts early) ----
        inp = ctx.enter_context(tc.tile_pool(name="inp", bufs=1))
        src_t = [inp.tile([P, M], bf16, tag=f"srcin{ci}", name=f"srcin{ci}")
                 for ci in range(2)]
        q_t = [inp.tile([P, NN], bf16, tag=f"qin{ci}", name=f"qin{ci}")
               for ci in range(2)]
        for c4 in range(8):       # src first (K then VT proj need it first),
            for ci in range(2):   # column-chunked so t=0 lands quickly,
                eng = nc.sync if ci == 0 else nc.scalar  # two DGE queues
                eng.dma_start(out=src_t[ci][:, c4 * 512:(c4 + 1) * 512],
                              in_=src_r[ci][:, c4 * 512:(c4 + 1) * 512])
        for c4 in range(4):
            for ci in range(2):
                nc.gpsimd.dma_start(out=q_t[ci][:, c4 * 512:(c4 + 1) * 512],
                                    in_=q_r[ci][:, c4 * 512:(c4 + 1) * 512])

        # ---- projections (bf16 inputs, fp32 PSUM accumulate) ----
        def proj_one(co, wt, xin, xlen, dst, bias):
            # c_out partitions, sequence on free dim; the PSUM->SBUF copies
            # with per-partition bias alternate ACT/DVE so neither engine
            # serializes the PE during projection phases. PSUM comes from
            # the scores pool so these matmuls never wait on the attention
            # drain holding the outT accumulators.
            for t in range(xlen // 512):
                ps = spool.tile([P, 1024], f32, tag="sc", name="ps")[:, 0:512]
                for ci in range(2):
                    nc.tensor.matmul(
                        ps[:],
                        wt[ci][:, co * P:(co + 1) * P],
                        xin[ci][:, t * 512:(t + 1) * 512],
                        start=(ci == 0), stop=(ci == 1),
                    )
                dslc = dst[co][:, t * 512:(t + 1) * 512]
                if t % 2 == 0:
                    nc.scalar.activation(out=dslc, in_=ps[:],
                                         func=Ident, bias=bias[co][:])
                else:
                    nc.vector.tensor_scalar_add(dslc, ps[:], bias[co])

        def proj_vt_chunk(mc):
            # VT = src.T @ WvT (m partitions, c_out free) + bv, stored as
            # per-head [VT_h | ones] blocks of width D+1
            ps = spool.tile([P, 1024], f32, tag="sc", name="psv")[:, 0:C]
            for ci in range(2):
                nc.tensor.matmul(
                    ps[:],
                    src_t[ci][:, mc * P:(mc + 1) * P],
                    wvt[ci][:],
                    start=(ci == 0), stop=(ci == 1),
                )
            nc.vector.tensor_tensor(
                vt[:, mc, :, 0:D],
                ps.rearrange("p (h d) -> p h d", h=H),
                bv_rep.rearrange("p (h d) -> p h d", h=H),
                add,
            )

        # ---- attention on head pairs ----
        def attention_pair(ch, nt):
            h0, h1 = 2 * ch, 2 * ch + 1
            n0 = nt * 1024
            outT = [opool.tile([D + 1, 1024], f32, tag="outT", name=f"oT{w}")
                    for w in range(2)]
            prs = {}
            for mc in range(MC + LAG):
                if mc < MC:
                    # MIXED score tiles: sc[ns] holds [h0_ns | h1_ns]. The
                    # two matmuls writing one tile share the same WAR
                    # dependency (the tile's previous exp), so the Tile
                    # scheduler keeps them adjacent — and on HW they run
                    # CONCURRENTLY (row-groups 0-63/64-127 via auto
                    # tile_position, different PSUM banks).
                    for ns in range(2):
                        sc = spool.tile([P, 1024], f32, tag="sc", name=f"sc{ns}")
                        for w, off in ((0, 0), (1, D)):
                            nc.tensor.matmul(
                                sc[:, w * 512:(w + 1) * 512],
                                K_sb[ch][off:off + D, mc * P:(mc + 1) * P],
                                Q_sb[ch][off:off + D,
                                         n0 + ns * 512:n0 + (ns + 1) * 512],
                                start=True, stop=True,
                            )
                        pr = probs_p.tile([P, 1024], bf16, tag="pr", name=f"pr{ns}")
                        if (mc + ns) % 2 == 0:
                            nc.vector.tensor_scalar(
                                pr[:].bitcast(i16), sc[:],
                                SCHR_A, SCHR_B, mult, add)
                        else:
                            nc.scalar.activation(
                                out=pr[:], in_=sc[:], func=Exp, scale=0.125)
                        prs[(mc, ns)] = pr
                if mc >= LAG:
                    j = mc - LAG
                    prA = prs.pop((j, 0))
                    prB = prs.pop((j, 1))
                    for w, h in ((0, h0), (1, h1)):
                        for ns, pr_j in ((0, prA), (1, prB)):
                            nc.tensor.matmul(
                                outT[w][:, ns * 512:(ns + 1) * 512],
                                vt[:, j, h, :],
                                pr_j[:, w * 512:(w + 1) * 512],
                                start=(j == 0), stop=(j == MC - 1),
                            )
            # drain: copy raw outT (incl. denominator row D) to SBUF right
            # away — one head on ACT, one on DVE — so the PSUM accumulators
            # free within the LAG window and the PE never stalls (a stalled
            # PE re-throttles the HAM clock 2.4->1.2 GHz). The recip/
            # broadcast/normalize chain then runs off the critical path.
            uout = [small_p.tile([D + 1, 1024], f32, tag=f"uo{w}", name=f"uo{w}")
                    for w in range(2)]
            nc.scalar.copy(out=uout[0][:], in_=outT[0][:])
            nc.vector.tensor_copy(out=uout[1][:], in_=outT[1][:])
            den = small_p.tile([1, 2048], f32, tag="den", name="den")
            for w in range(2):
                nc.vector.tensor_copy(out=den[0:1, w * 1024:(w + 1) * 1024],
                                      in_=uout[w][D:D + 1, :])
            rec = small_p.tile([1, 2048], f32, tag="rec", name="rec")
            scr = small_p.tile([1, 2048], f32, tag="scr", name="scr")
            nc.vector.reciprocal_approx_accurate(
                out=rec[0:1, :], in_=den[0:1, :], scratch=scr[0:1, :])
            dscr = dram_p.tile([1, 2048], f32, name="dscr")
            nc.sync.dma_start(out=dscr[:], in_=rec[0:1, :])
            for w, h in ((0, h0), (1, h1)):
                row = dscr[0:1, w * 1024:(w + 1) * 1024]
                rrep = small_p.tile([D, 1024], f32, tag=f"rrep{w}", name=f"rrep{w}")
                nc.sync.dma_start(
                    out=rrep[:],
                    in_=bass.AP(tensor=row.tensor, offset=row.offset,
                                ap=[[0, D]] + list(row.ap)[1:]))
                nc.vector.tensor_tensor(
                    attn[h][:, n0:n0 + 1024],
                    uout[w][0:D, :],
                    rrep[:],
                    mult,
                )

        def merge_nt(nt):
            for co in range(2):
                for t in range(2 * nt, 2 * nt + 2):
                    ps = spool.tile([P, 1024], f32, tag="sc", name="psm")[:, 0:512]
                    for h in range(H):
                        nc.tensor.matmul(
                            ps[:],
                            wm_h[h][:, co * P:(co + 1) * P],
                            attn[h][:, t * 512:(t + 1) * 512],
                            start=(h == 0), stop=(h == H - 1),
                        )
                    ot = ostage.tile([P, 512], f32, tag="ot", name="ot")
                    if t % 2 == 0:
                        nc.scalar.activation(out=ot[:], in_=ps[:],
                                             func=Ident, bias=bm_t[co][:])
                    else:
                        nc.vector.tensor_scalar_add(ot[:], ps[:], bm_t[co])
                    nc.sync.dma_start(out=y_r[co, :, t * 512:(t + 1) * 512],
                                      in_=ot[:])

        proj_one(0, wkt, src_t, M, K_sb, bk_t)
        proj_one(0, wqt, q_t, NN, Q_sb, bq_t)
        for mc in range(MC):
            proj_vt_chunk(mc)
        attention_pair(0, 0)
        attention_pair(0, 1)
        proj_one(1, wkt, src_t, M, K_sb, bk_t)
        proj_one(1, wqt, q_t, NN, Q_sb, bq_t)
        attention_pair(1, 0)
        merge_nt(0)          # heads complete for nt=0 here; fills the
        attention_pair(1, 1)  # pair(1,1) ramp and shortens the tail
        merge_nt(1)

    nc.compile()
    return nc


def _get_nc():
    if "nc" not in _STATE:
        _STATE["nc"] = _build()
    return _STATE["nc"]


def kernel(query, source, Wq, bq, Wk, bk, Wv, bv, Wm, bm):
    import ml_dtypes
    from concourse.bass_utils import run_bass_kernel_spmd

    bf16 = ml_dtypes.bfloat16
    query = np.asarray(query, np.float32).astype(bf16)
    source = np.asarray(source, np.float32).astype(bf16)
    wqT = np.ascontiguousarray(np.asarray(Wq, np.float32).T).astype(bf16)
    wkT = np.ascontiguousarray(np.asarray(Wk, np.float32).T).astype(bf16)
    wvT = np.ascontiguousarray(np.asarray(Wv, np.float32).T).astype(bf16)
    wmT = np.ascontiguousarray(np.asarray(Wm, np.float32).T).astype(bf16)
    bq = np.asarray(bq, np.float32)
    bk = np.asarray(bk, np.float32)
    bv = np.asarray(bv, np.float32)
    bm = np.asarray(bm, np.float32)

    nc = _get_nc()

    in_maps = []
    for c in range(N_CORES):
        b, nh = c // 2, c % 2
        in_maps.append({
            "q": np.ascontiguousarray(query[b, :, nh * NN:(nh + 1) * NN]),
            "src": np.ascontiguousarray(source[b]),
            "wqT": wqT, "wkT": wkT, "wvT": wvT, "wmT": wmT,
            "bq": bq, "bk": bk, "bv": bv, "bm": bm,
        })

    trace = os.environ.get("KERNEL_TRACE") == "1"
    res = run_bass_kernel_spmd(
        nc, in_maps, core_ids=list(range(N_CORES)), trace=trace)
    _STATE["last_result"] = res
    if trace and res.exec_time_ns is not None:
        print(f"HW exec time: {res.exec_time_ns} ns")

    out = np.empty((B, C, N), np.float32)
    for c in range(N_CORES):
        b, nh = c // 2, c % 2
        out[b, :, nh * NN:(nh + 1) * NN] = res.results[c]["y"]
    return out


# revision 19
# speedup vs baseline: 1.3443x; 1.1059x over previous
"""MultiHeadAttention kernel for Trainium2, 8 NeuronCores.

Problem shapes (hardcoded): B=4, C=256, N=M=4096, H=4 heads, D=64 head dim.
reference: Q/K/V = 1x1-conv projections, scores = Q^T K / sqrt(D) per head,
softmax over source dim, out = attn @ V^T, merge projection.

Sharding: 8 cores = (batch b, query-half nh). Each core computes the full
output rows for its (b, n-half): K/V projections are recomputed per n-half
(5% redundant FLOPs) which keeps every core's output slice disjoint — the
host just concatenates, no reductions.

Per-core dataflow (bf16 matmul operands, fp32 PSUM accumulation):
  K  = WkT.T @ src            (c_out on partitions, m free)    [PE]
  Q  = WqT.T @ q              (c_out on partitions, n free)    [PE]
  VT = src.T @ WvT            (m on partitions, c_out free)    [PE]
  attention runs on HEAD PAIRS (2ch, 2ch+1): their score matmuls contract
  only D=64, so the pair occupies PE row-groups 0-63 / 64-127 (tile_position
  auto-derived from base_partition) and the two matmuls execute
  CONCURRENTLY in the systolic array — scores PE time ~halves.
  per (pair ch, n-tile of 1024, m-chunk of 128):
    scoresT_h[m,n] = K_h^T Q_h  both heads, row-tiled          [PE]
    probs = exp(scores/8): one head's chunk on ACT (LUT exp), the
    other on DVE via a bf16 Schraudolph bit-trick:
      bits16 = round(A*s + B), A = 128*log2(e)/8, viewed as bf16
    alternating per m-chunk so every softmax row mixes both paths
    (rel err ~7e-3 end to end)                                 [ACT+DVE]
    outT_h(65,1024) += probs_chunk.T @ [VT_h | ones]  -> row 64 is
    the softmax denominator                                    [PE]
  pair drain: denominators batched (2,1024) -> approx-NR recip [DVE],
    partition-broadcast via a DRAM bounce [DMA], attn_h = outT * r [DVE]
  y = WmT.T @ attn + bm       contract heads, K=64 each        [PE]

Engine balance (the previous version was ACT(exp)-bound at ~343us busy):
exp is split ~50/50 ACT/DVE; K/Q/merge PSUM->SBUF bias-copies run on ACT
(activation Copy with per-partition bias AP); VT bias-adds + attn
normalization stay on DVE. PE does ~200us, ACT/DVE ~200us each.

Hardware landmines (kept working around, see git history of this file):
  - gpsimd.partition_broadcast reads the wrong partition for inputs not
    based at partition 0, and heavy gpsimd SBUF traffic locks the
    DVE-shared port;
  - DMA with partition-shifted or partition-step-0 SBUF APs hangs the
    device (DRAM-side broadcast APs are fine);
  - DVE ops are partition-locked (out/in must share the partition base),
    though a plain reciprocal/copy CAN shift base; the custom-DVE
    reciprocal_approx ops cannot;
  - matmul out must stay within one PSUM bank (<=512 fp32 free).
"""

import os

import numpy as np

N_CORES = 8
B, C = 4, 256
N = M = 4096
H, D = 4, 64
NN = N // 2          # query positions per core
P = 128
NT = NN // 1024      # n-tiles per core (2)
MC = M // P          # m-chunks (32)
LAG = 4              # outT matmuls trail scores by LAG m-chunks

# Schraudolph bf16 exp: bits16 = round(SCHR_A * s + SCHR_B); includes the
# 1/sqrt(D)=0.125 score scaling. B centers the log-error (C_adj ~ 7.5).
SCHR_A = float(128.0 * 0.125 / np.log(2.0))
SCHR_B = float(128.0 * 127.0 - 7.5)

_STATE: dict = {}


def _build():
    from contextlib import ExitStack

    import concourse.bass as bass
    import concourse.mybir as mybir
    import concourse.tile as tile
    from concourse import bacc

    f32 = mybir.dt.float32
    bf16 = mybir.dt.bfloat16
    i16 = mybir.dt.int16
    Exp = mybir.ActivationFunctionType.Exp
    Ident = mybir.ActivationFunctionType.Identity
    add = mybir.AluOpType.add
    mult = mybir.AluOpType.mult

    nc = bacc.Bacc(
        "TRN2",
        target_bir_lowering=False,
        debug=False,
        enable_asserts=False,
        num_devices=N_CORES,
    )

    q_d = nc.dram_tensor("q", (C, NN), bf16, kind="ExternalInput").ap()
    src_d = nc.dram_tensor("src", (C, M), bf16, kind="ExternalInput").ap()
    wqT_d = nc.dram_tensor("wqT", (C, C), bf16, kind="ExternalInput").ap()
    wkT_d = nc.dram_tensor("wkT", (C, C), bf16, kind="ExternalInput").ap()
    wvT_d = nc.dram_tensor("wvT", (C, C), bf16, kind="ExternalInput").ap()
    wmT_d = nc.dram_tensor("wmT", (C, C), bf16, kind="ExternalInput").ap()
    bq_d = nc.dram_tensor("bq", (C,), f32, kind="ExternalInput").ap()
    bk_d = nc.dram_tensor("bk", (C,), f32, kind="ExternalInput").ap()
    bv_d = nc.dram_tensor("bv", (C,), f32, kind="ExternalInput").ap()
    bm_d = nc.dram_tensor("bm", (C,), f32, kind="ExternalInput").ap()
    y_d = nc.dram_tensor("y", (C, NN), f32, kind="ExternalOutput").ap()

    q_r = q_d.rearrange("(a p) n -> a p n", p=P)
    src_r = src_d.rearrange("(a p) n -> a p n", p=P)
    y_r = y_d.rearrange("(a p) n -> a p n", p=P)

    def chunks(w):
        return w.rearrange("(a p) n -> a p n", p=P)

    with tile.TileContext(nc) as tc, ExitStack() as ctx:
        singles = ctx.enter_context(tc.tile_pool(name="singles", bufs=1))
        # PSUM: scores 2 tiles x 2 banks + outT (two live head
        # accumulators) 2 x 2 banks = 8 banks. Projections/VT borrow the
        # outT slots (they run strictly outside the attention pairs).
        spool = ctx.enter_context(tc.tile_pool(name="scores", bufs=2, space="PSUM"))
        opool = ctx.enter_context(tc.tile_pool(name="outps", bufs=2, space="PSUM"))
        probs_p = ctx.enter_context(tc.tile_pool(name="probs", bufs=2 * (LAG + 1)))
        small_p = ctx.enter_context(tc.tile_pool(name="small", bufs=2))
        dram_p = ctx.enter_context(tc.tile_pool(name="dram", bufs=2, space="DRAM"))
        ostage = ctx.enter_context(tc.tile_pool(name="ostage", bufs=3))

        # ---- weights / biases ----
        wqt, wkt, wvt = [], [], []
        for ci in range(2):
            for lst, d, nm in ((wqt, wqT_d, "wq"), (wkt, wkT_d, "wk"),
                               (wvt, wvT_d, "wv")):
                t = singles.tile([P, C], bf16, tag=f"{nm}{ci}", name=f"{nm}{ci}")
                nc.gpsimd.dma_start(out=t[:], in_=chunks(d)[ci])
                lst.append(t)
        wm_h = []
        for h in range(H):
            t = singles.tile([D, C], bf16, tag=f"wm{h}", name=f"wm{h}")
            nc.gpsimd.dma_start(out=t[:], in_=wmT_d[h * D:(h + 1) * D, :])
            wm_h.append(t)
        bq_t, bk_t, bm_t = [], [], []
        for ci in range(2):
            for lst, d, nm in ((bq_t, bq_d, "bq"), (bk_t, bk_d, "bk"),
                               (bm_t, bm_d, "bm")):
                t = singles.tile([P, 1], f32, tag=f"{nm}{ci}", name=f"b{nm}{ci}")
                nc.gpsimd.dma_start(out=t[:], in_=d.rearrange("(a p) -> a p", p=P)[ci][:, None])
                lst.append(t)
        bv_rep = singles.tile([P, C], f32, tag="bv_rep", name="bv_rep")
        nc.gpsimd.dma_start(
            out=bv_rep[:],
            in_=bass.AP(tensor=bv_d.tensor, offset=bv_d.offset,
                        ap=[[0, P]] + list(bv_d.ap)),
        )

        # ---- persistent activations ----
        Q_sb = [singles.tile([P, NN], bf16, tag=f"Q{ci}", name=f"Q{ci}") for ci in range(2)]
        K_sb = [singles.tile([P, M], bf16, tag=f"K{ci}", name=f"K{ci}") for ci in range(2)]
        vt = singles.tile([P, MC, H, D + 1], bf16, tag="vt", name="vt")
        attn = [singles.tile([D, NN], bf16, tag=f"attn{h}", name=f"attn{h}") for h in range(H)]

        nc.vector.memset(vt[:, :, :, D:D + 1], 1.0)

        # ---- inputs (chunked DMAs so the first matmul starts early) ----
        inp = ctx.enter_context(tc.tile_pool(name="inp", bufs=1))
        src_t = [inp.tile([P, M], bf16, tag=f"srcin{ci}", name=f"srcin{ci}")
                 for ci in range(2)]
        q_t = [inp.tile([P, NN], bf16, tag=f"qin{ci}", name=f"qin{ci}")
               for ci in range(2)]
        for c4 in range(8):       # src first (K then VT proj need it first),
            for ci in range(2):   # column-chunked so t=0 lands quickly,
                eng = nc.sync if ci == 0 else nc.scalar  # two DGE queues
                eng.dma_start(out=src_t[ci][:, c4 * 512:(c4 + 1) * 512],
                              in_=src_r[ci][:, c4 * 512:(c4 + 1) * 512])
        for c4 in range(4):
            for ci in range(2):
                nc.gpsimd.dma_start(out=q_t[ci][:, c4 * 512:(c4 + 1) * 512],
                                    in_=q_r[ci][:, c4 * 512:(c4 + 1) * 512])

        # ---- projections (bf16 inputs, fp32 PSUM accumulate) ----
        def proj_one(co, wt, xin, xlen, dst, bias):
            # c_out partitions, sequence on free dim; the PSUM->SBUF copies
            # with per-partition bias alternate ACT/DVE so neither engine
            # serializes the PE during projection phases. PSUM comes from
            # the scores pool so these matmuls never wait on the attention
            # drain holding the outT accumulators.
            for t in range(xlen // 512):
                ps = spool.tile([P, 1024], f32, tag="sc", name="ps")[:, 0:512]
                for ci in range(2):
                    nc.tensor.matmul(
                        ps[:],
                        wt[ci][:, co * P:(co + 1) * P],
                        xin[ci][:, t * 512:(t + 1) * 512],
                        start=(ci == 0), stop=(ci == 1),
                    )
                dslc = dst[co][:, t * 512:(t + 1) * 512]
                if t % 2 == 0:
                    nc.scalar.activation(out=dslc, in_=ps[:],
                                         func=Ident, bias=bias[co][:])
                else:
                    nc.vector.tensor_scalar_add(dslc, ps[:], bias[co])

        def proj_vt_chunk(mc):
            # VT = src.T @ WvT (m partitions, c_out free) + bv, stored as
            # per-head [VT_h | ones] blocks of width D+1
            ps = spool.tile([P, 1024], f32, tag="sc", name="psv")[:, 0:C]
            for ci in range(2):
                nc.tensor.matmul(
                    ps[:],
                    src_t[ci][:, mc * P:(mc + 1) * P],
                    wvt[ci][:],
                    start=(ci == 0), stop=(ci == 1),
                )
            nc.vector.tensor_tensor(
                vt[:, mc, :, 0:D],
                ps.rearrange("p (h d) -> p h d", h=H),
                bv_rep.rearrange("p (h d) -> p h d", h=H),
                add,
            )

        # ---- attention on head pairs ----
        def attention_pair(ch, nt):
            h0, h1 = 2 * ch, 2 * ch + 1
            n0 = nt * 1024
            outT = [opool.tile([D + 1, 1024], f32, tag="outT", name=f"oT{w}")
                    for w in range(2)]
            prs = {}
            for mc in range(MC + LAG):
                if mc < MC:
                    # MIXED score tiles: sc[ns] holds [h0_ns | h1_ns]. The
                    # two matmuls writing one tile share the same WAR
                    # dependency (the tile's previous exp), so the Tile
                    # scheduler keeps them adjacent — and on HW they run
                    # CONCURRENTLY (row-groups 0-63/64-127 via auto
                    # tile_position, different PSUM banks).
                    for ns in range(2):
                        sc = spool.tile([P, 1024], f32, tag="sc", name=f"sc{ns}")
                        for w, off in ((0, 0), (1, D)):
                            nc.tensor.matmul(
                                sc[:, w * 512:(w + 1) * 512],
                                K_sb[ch][off:off + D, mc * P:(mc + 1) * P],
                                Q_sb[ch][off:off + D,
                                         n0 + ns * 512:n0 + (ns + 1) * 512],
                                start=True, stop=True,
                            )
                        pr = probs_p.tile([P, 1024], bf16, tag="pr", name=f"pr{ns}")
                        # ~41% of chunks on DVE (13 of 32), the rest on ACT:
                        # balances engine time (DVE also carries the
                        # normalize/recip fixed work); spread so every
                        # softmax row mixes both exp paths
                        if ((2 * mc + ns) * 13) % 32 < 13:
                            nc.vector.tensor_scalar(
                                pr[:].bitcast(i16), sc[:],
                                SCHR_A, SCHR_B, mult, add)
                        else:
                            nc.scalar.activation(
                                out=pr[:], in_=sc[:], func=Exp, scale=0.125)
                        prs[(mc, ns)] = pr
                if mc >= LAG:
                    j = mc - LAG
                    prA = prs.pop((j, 0))
                    prB = prs.pop((j, 1))
                    for w, h in ((0, h0), (1, h1)):
                        for ns, pr_j in ((0, prA), (1, prB)):
                            nc.tensor.matmul(
                                outT[w][:, ns * 512:(ns + 1) * 512],
                                vt[:, j, h, :],
                                pr_j[:, w * 512:(w + 1) * 512],
                                start=(j == 0), stop=(j == MC - 1),
                            )
            # drain: copy raw outT (incl. denominator row D) to SBUF right
            # away — one head on ACT, one on DVE — so the PSUM accumulators
            # free within the LAG window and the PE never stalls (a stalled
            # PE re-throttles the HAM clock 2.4->1.2 GHz). The recip/
            # broadcast/normalize chain then runs off the critical path.
            uout = [small_p.tile([D + 1, 1024], f32, tag=f"uo{w}", name=f"uo{w}")
                    for w in range(2)]
            nc.scalar.copy(out=uout[0][:], in_=outT[0][:])
            nc.vector.tensor_copy(out=uout[1][:], in_=outT[1][:])
            den = small_p.tile([1, 2048], f32, tag="den", name="den")
            for w in range(2):
                nc.vector.tensor_copy(out=den[0:1, w * 1024:(w + 1) * 1024],
                                      in_=uout[w][D:D + 1, :])
            rec = small_p.tile([1, 2048], f32, tag="rec", name="rec")
            nc.vector.reciprocal_approx_fast(out=rec[0:1, :], in_=den[0:1, :])
            dscr = dram_p.tile([1, 2048], f32, name="dscr")
            nc.sync.dma_start(out=dscr[:], in_=rec[0:1, :])
            for w, h in ((0, h0), (1, h1)):
                row = dscr[0:1, w * 1024:(w + 1) * 1024]
                rrep = small_p.tile([D, 1024], f32, tag=f"rrep{w}", name=f"rrep{w}")
                nc.sync.dma_start(
                    out=rrep[:],
                    in_=bass.AP(tensor=row.tensor, offset=row.offset,
                                ap=[[0, D]] + list(row.ap)[1:]))
                nc.vector.tensor_tensor(
                    attn[h][:, n0:n0 + 1024],
                    uout[w][0:D, :],
                    rrep[:],
                    mult,
                )

        def merge_nt(nt):
            for co in range(2):
                for t in range(2 * nt, 2 * nt + 2):
                    ps = spool.tile([P, 1024], f32, tag="sc", name="psm")[:, 0:512]
                    for h in range(H):
                        nc.tensor.matmul(
                            ps[:],
                            wm_h[h][:, co * P:(co + 1) * P],
                            attn[h][:, t * 512:(t + 1) * 512],
                            start=(h == 0), stop=(h == H - 1),
                        )
                    ot = ostage.tile([P, 512], f32, tag="ot", name="ot")
                    if t % 2 == 0:
                        nc.scalar.activation(out=ot[:], in_=ps[:],
                                             func=Ident, bias=bm_t[co][:])
                    else:
                        nc.vector.tensor_scalar_add(ot[:], ps[:], bm_t[co])
                    nc.sync.dma_start(out=y_r[co, :, t * 512:(t + 1) * 512],
                                      in_=ot[:])

        proj_one(0, wkt, src_t, M, K_sb, bk_t)
        proj_one(0, wqt, q_t, NN, Q_sb, bq_t)
        for mc in range(MC):
            proj_vt_chunk(mc)
        attention_pair(0, 0)
        attention_pair(0, 1)
        proj_one(1, wkt, src_t, M, K_sb, bk_t)
        proj_one(1, wqt, q_t, NN, Q_sb, bq_t)
        attention_pair(1, 0)
        attention_pair(1, 1)
        merge_nt(0)
        merge_nt(1)

    nc.compile()
    return nc


def _get_nc():
    if "nc" not in _STATE:
        _STATE["nc"] = _build()
    return _STATE["nc"]


def kernel(query, source, Wq, bq, Wk, bk, Wv, bv, Wm, bm):
    import ml_dtypes
    from concourse.bass_utils import run_bass_kernel_spmd

    bf16 = ml_dtypes.bfloat16
    query = np.asarray(query, np.float32).astype(bf16)
    source = np.asarray(source, np.float32).astype(bf16)
    wqT = np.ascontiguousarray(np.asarray(Wq, np.float32).T).astype(bf16)
    wkT = np.ascontiguousarray(np.asarray(Wk, np.float32).T).astype(bf16)
    wvT = np.ascontiguousarray(np.asarray(Wv, np.float32).T).astype(bf16)
    wmT = np.ascontiguousarray(np.asarray(Wm, np.float32).T).astype(bf16)
    bq = np.asarray(bq, np.float32)
    bk = np.asarray(bk, np.float32)
    bv = np.asarray(bv, np.float32)
    bm = np.asarray(bm, np.float32)

    nc = _get_nc()

    in_maps = []
    for c in range(N_CORES):
        b, nh = c // 2, c % 2
        in_maps.append({
            "q": np.ascontiguousarray(query[b, :, nh * NN:(nh + 1) * NN]),
            "src": np.ascontiguousarray(source[b]),
            "wqT": wqT, "wkT": wkT, "wvT": wvT, "wmT": wmT,
            "bq": bq, "bk": bk, "bv": bv, "bm": bm,
        })

    trace = os.environ.get("KERNEL_TRACE") == "1"
    res = run_bass_kernel_spmd(
        nc, in_maps, core_ids=list(range(N_CORES)), trace=trace)
    _STATE["last_result"] = res
    if trace and res.exec_time_ns is not None:
        print(f"HW exec time: {res.exec_time_ns} ns")

    out = np.empty((B, C, N), np.float32)
    for c in range(N_CORES):
        b, nh = c // 2, c % 2
        out[b, :, nh * NN:(nh + 1) * NN] = res.results[c]["y"]
    return out


# revision 23
# speedup vs baseline: 1.4181x; 1.0549x over previous
"""MultiHeadAttention kernel for Trainium2, 8 NeuronCores.

Problem shapes (hardcoded): B=4, C=256, N=M=4096, H=4 heads, D=64 head dim.
reference: Q/K/V = 1x1-conv projections, scores = Q^T K / sqrt(D) per head,
softmax over source dim, out = attn @ V^T, merge projection.

Sharding: 8 cores = (batch b, query-half nh). Each core computes the full
output rows for its (b, n-half): K/V projections are recomputed per n-half
(5% redundant FLOPs) which keeps every core's output slice disjoint — the
host just concatenates, no reductions.

Per-core dataflow (bf16 matmul operands, fp32 PSUM accumulation):
  K  = WkT.T @ src            (c_out on partitions, m free)    [PE]
  Q  = WqT.T @ q              (c_out on partitions, n free)    [PE]
  VT = src.T @ WvT            (m on partitions, c_out free)    [PE]
  attention runs on HEAD PAIRS (2ch, 2ch+1): their score matmuls contract
  only D=64, so the pair occupies PE row-groups 0-63 / 64-127 (tile_position
  auto-derived from base_partition) and the two matmuls execute
  CONCURRENTLY in the systolic array — scores PE time ~halves.
  per (pair ch, n-tile of 1024, m-chunk of 128):
    scoresT_h[m,n] = K_h^T Q_h  both heads, row-tiled          [PE]
    probs = exp(scores/8): one head's chunk on ACT (LUT exp), the
    other on DVE via a bf16 Schraudolph bit-trick:
      bits16 = round(A*s + B), A = 128*log2(e)/8, viewed as bf16
    alternating per m-chunk so every softmax row mixes both paths
    (rel err ~7e-3 end to end)                                 [ACT+DVE]
    outT_h(65,1024) += probs_chunk.T @ [VT_h | ones]  -> row 64 is
    the softmax denominator                                    [PE]
  pair drain: denominators batched (2,1024) -> approx-NR recip [DVE],
    partition-broadcast via a DRAM bounce [DMA], attn_h = outT * r [DVE]
  y = WmT.T @ attn + bm       contract heads, K=64 each        [PE]

Engine balance (the previous version was ACT(exp)-bound at ~343us busy):
exp is split ~50/50 ACT/DVE; K/Q/merge PSUM->SBUF bias-copies run on ACT
(activation Copy with per-partition bias AP); VT bias-adds + attn
normalization stay on DVE. PE does ~200us, ACT/DVE ~200us each.

Hardware landmines (kept working around, see git history of this file):
  - gpsimd.partition_broadcast reads the wrong partition for inputs not
    based at partition 0, and heavy gpsimd SBUF traffic locks the
    DVE-shared port;
  - DMA with partition-shifted or partition-step-0 SBUF APs hangs the
    device (DRAM-side broadcast APs are fine);
  - DVE ops are partition-locked (out/in must share the partition base),
    though a plain reciprocal/copy CAN shift base; the custom-DVE
    reciprocal_approx ops cannot;
  - matmul out must stay within one PSUM bank (<=512 fp32 free).
"""

import os

import numpy as np

N_CORES = 8
B, C = 4, 256
N = M = 4096
H, D = 4, 64
NN = N // 2          # query positions per core
P = 128
NT = NN // 512       # n-tiles per core (4)
MC = M // P          # m-chunks (32)
LAG = 4              # outT matmuls trail scores by LAG m-chunks

# Schraudolph bf16 exp: bits16 = round(SCHR_A * s + SCHR_B); includes the
# 1/sqrt(D)=0.125 score scaling. B centers the log-error (C_adj ~ 7.5).
SCHR_A = float(128.0 * 0.125 / np.log(2.0))
SCHR_B = float(128.0 * 127.0 - 7.5)

_STATE: dict = {}


def _build():
    from contextlib import ExitStack

    import concourse.bass as bass
    import concourse.mybir as mybir
    import concourse.tile as tile
    from concourse import bacc

    f32 = mybir.dt.float32
    bf16 = mybir.dt.bfloat16
    i16 = mybir.dt.int16
    Exp = mybir.ActivationFunctionType.Exp
    Ident = mybir.ActivationFunctionType.Identity
    add = mybir.AluOpType.add
    mult = mybir.AluOpType.mult

    nc = bacc.Bacc(
        "TRN2",
        target_bir_lowering=False,
        debug=False,
        enable_asserts=False,
        num_devices=N_CORES,
    )

    q_d = nc.dram_tensor("q", (C, NN), bf16, kind="ExternalInput").ap()
    src_d = nc.dram_tensor("src", (C, M), bf16, kind="ExternalInput").ap()
    wqT_d = nc.dram_tensor("wqT", (C, C), bf16, kind="ExternalInput").ap()
    wkT_d = nc.dram_tensor("wkT", (C, C), bf16, kind="ExternalInput").ap()
    wvT_d = nc.dram_tensor("wvT", (C, C), bf16, kind="ExternalInput").ap()
    wmT_d = nc.dram_tensor("wmT", (C, C), bf16, kind="ExternalInput").ap()
    bq_d = nc.dram_tensor("bq", (C,), f32, kind="ExternalInput").ap()
    bk_d = nc.dram_tensor("bk", (C,), f32, kind="ExternalInput").ap()
    bv_d = nc.dram_tensor("bv", (C,), f32, kind="ExternalInput").ap()
    bm_d = nc.dram_tensor("bm", (C,), f32, kind="ExternalInput").ap()
    y_d = nc.dram_tensor("y", (C, NN), f32, kind="ExternalOutput").ap()

    q_r = q_d.rearrange("(a p) n -> a p n", p=P)
    src_r = src_d.rearrange("(a p) n -> a p n", p=P)
    y_r = y_d.rearrange("(a p) n -> a p n", p=P)

    def chunks(w):
        return w.rearrange("(a p) n -> a p n", p=P)

    with tile.TileContext(nc) as tc, ExitStack() as ctx:
        singles = ctx.enter_context(tc.tile_pool(name="singles", bufs=1))
        # PSUM: scores 3 tiles x 2 banks (3-deep rotation so score matmuls
        # never WAR-wait on the exp of the previous chunk) + outT (two live
        # (65,512) head accumulators) 2 x 1 bank = 8 banks.
        spool = ctx.enter_context(tc.tile_pool(name="scores", bufs=3, space="PSUM"))
        opool = ctx.enter_context(tc.tile_pool(name="outps", bufs=2, space="PSUM"))
        probs_p = ctx.enter_context(tc.tile_pool(name="probs", bufs=LAG + 2))
        small_p = ctx.enter_context(tc.tile_pool(name="small", bufs=2))
        dram_p = ctx.enter_context(tc.tile_pool(name="dram", bufs=2, space="DRAM"))
        ostage = ctx.enter_context(tc.tile_pool(name="ostage", bufs=3))

        # ---- weights / biases ----
        wqt, wkt, wvt = [], [], []
        for ci in range(2):
            for lst, d, nm in ((wqt, wqT_d, "wq"), (wkt, wkT_d, "wk"),
                               (wvt, wvT_d, "wv")):
                t = singles.tile([P, C], bf16, tag=f"{nm}{ci}", name=f"{nm}{ci}")
                nc.gpsimd.dma_start(out=t[:], in_=chunks(d)[ci])
                lst.append(t)
        wm_h = []
        for h in range(H):
            t = singles.tile([D, C], bf16, tag=f"wm{h}", name=f"wm{h}")
            nc.gpsimd.dma_start(out=t[:], in_=wmT_d[h * D:(h + 1) * D, :])
            wm_h.append(t)
        bq_t, bk_t, bm_t = [], [], []
        for ci in range(2):
            for lst, d, nm in ((bq_t, bq_d, "bq"), (bk_t, bk_d, "bk"),
                               (bm_t, bm_d, "bm")):
                t = singles.tile([P, 1], f32, tag=f"{nm}{ci}", name=f"b{nm}{ci}")
                nc.gpsimd.dma_start(out=t[:], in_=d.rearrange("(a p) -> a p", p=P)[ci][:, None])
                lst.append(t)
        bv_rep = singles.tile([P, C], f32, tag="bv_rep", name="bv_rep")
        nc.gpsimd.dma_start(
            out=bv_rep[:],
            in_=bass.AP(tensor=bv_d.tensor, offset=bv_d.offset,
                        ap=[[0, P]] + list(bv_d.ap)),
        )

        # ---- persistent activations ----
        Q_sb = [singles.tile([P, NN], bf16, tag=f"Q{ci}", name=f"Q{ci}") for ci in range(2)]
        K_sb = [singles.tile([P, M], bf16, tag=f"K{ci}", name=f"K{ci}") for ci in range(2)]
        vt = singles.tile([P, MC, H, D + 1], bf16, tag="vt", name="vt")
        attn = [singles.tile([D, NN], bf16, tag=f"attn{h}", name=f"attn{h}") for h in range(H)]

        nc.vector.memset(vt[:, :, :, D:D + 1], 1.0)

        # ---- inputs (chunked DMAs so the first matmul starts early) ----
        inp = ctx.enter_context(tc.tile_pool(name="inp", bufs=1))
        src_t = [inp.tile([P, M], bf16, tag=f"srcin{ci}", name=f"srcin{ci}")
                 for ci in range(2)]
        q_t = [inp.tile([P, NN], bf16, tag=f"qin{ci}", name=f"qin{ci}")
               for ci in range(2)]
        for c4 in range(8):       # src first (K then VT proj need it first),
            for ci in range(2):   # column-chunked so t=0 lands quickly,
                eng = nc.sync if ci == 0 else nc.scalar  # two DGE queues
                eng.dma_start(out=src_t[ci][:, c4 * 512:(c4 + 1) * 512],
                              in_=src_r[ci][:, c4 * 512:(c4 + 1) * 512])
        for c4 in range(4):
            for ci in range(2):
                nc.gpsimd.dma_start(out=q_t[ci][:, c4 * 512:(c4 + 1) * 512],
                                    in_=q_r[ci][:, c4 * 512:(c4 + 1) * 512])

        # ---- projections (bf16 inputs, fp32 PSUM accumulate) ----
        def proj_one(co, wt, xin, xlen, dst, bias):
            # c_out partitions, sequence on free dim; the PSUM->SBUF copies
            # with per-partition bias alternate ACT/DVE so neither engine
            # serializes the PE during projection phases. PSUM comes from
            # the scores pool so these matmuls never wait on the attention
            # drain holding the outT accumulators.
            for t in range(xlen // 512):
                ps = spool.tile([P, 1024], f32, tag="sc", name="ps")[:, 0:512]
                for ci in range(2):
                    nc.tensor.matmul(
                        ps[:],
                        wt[ci][:, co * P:(co + 1) * P],
                        xin[ci][:, t * 512:(t + 1) * 512],
                        start=(ci == 0), stop=(ci == 1),
                    )
                dslc = dst[co][:, t * 512:(t + 1) * 512]
                if t % 2 == 0:
                    nc.scalar.activation(out=dslc, in_=ps[:],
                                         func=Ident, bias=bias[co][:])
                else:
                    nc.vector.tensor_scalar_add(dslc, ps[:], bias[co])

        def proj_vt_chunk(mc):
            # VT = src.T @ WvT (m partitions, c_out free) + bv, stored as
            # per-head [VT_h | ones] blocks of width D+1
            ps = spool.tile([P, 1024], f32, tag="sc", name="psv")[:, 0:C]
            for ci in range(2):
                nc.tensor.matmul(
                    ps[:],
                    src_t[ci][:, mc * P:(mc + 1) * P],
                    wvt[ci][:],
                    start=(ci == 0), stop=(ci == 1),
                )
            nc.vector.tensor_tensor(
                vt[:, mc, :, 0:D],
                ps.rearrange("p (h d) -> p h d", h=H),
                bv_rep.rearrange("p (h d) -> p h d", h=H),
                add,
            )

        # ---- attention on head pairs, n-tiles of 512 ----
        exp_slot = [0]

        def attention_pair(ch, nt):
            h0, h1 = 2 * ch, 2 * ch + 1
            n0 = nt * 512
            outT = [opool.tile([D + 1, 512], f32, tag="outT", name=f"oT{w}")
                    for w in range(2)]
            prs = {}
            for mc in range(MC + LAG):
                if mc < MC:
                    # MIXED score tile: [h0 | h1] halves. The two matmuls
                    # writing it share the same WAR dependency (the tile's
                    # previous exp) so the Tile scheduler keeps them
                    # adjacent — and on HW they run CONCURRENTLY
                    # (row-groups 0-63/64-127 via auto tile_position,
                    # different PSUM banks).
                    sc = spool.tile([P, 1024], f32, tag="sc", name="sc")
                    for w, off in ((0, 0), (1, D)):
                        nc.tensor.matmul(
                            sc[:, w * 512:(w + 1) * 512],
                            K_sb[ch][off:off + D, mc * P:(mc + 1) * P],
                            Q_sb[ch][off:off + D, n0:n0 + 512],
                            start=True, stop=True,
                        )
                    pr = probs_p.tile([P, 1024], bf16, tag="pr", name="pr")
                    # ~41% of chunks on DVE (13 of 32), the rest on ACT:
                    # balances engine time (DVE also carries the normalize/
                    # recip fixed work); the pattern spreads both exp paths
                    # across every softmax row's m-chunks
                    s = exp_slot[0]
                    exp_slot[0] += 1
                    if (s * 13) % 32 < 13:
                        nc.vector.tensor_scalar(
                            pr[:].bitcast(i16), sc[:],
                            SCHR_A, SCHR_B, mult, add)
                    else:
                        nc.scalar.activation(
                            out=pr[:], in_=sc[:], func=Exp, scale=0.125)
                    prs[mc] = pr
                if mc >= LAG:
                    j = mc - LAG
                    pr_j = prs.pop(j)
                    for w, h in ((0, h0), (1, h1)):
                        nc.tensor.matmul(
                            outT[w][:],
                            vt[:, j, h, :],
                            pr_j[:, w * 512:(w + 1) * 512],
                            start=(j == 0), stop=(j == MC - 1),
                        )
            # drain: copy raw outT (incl. denominator row D) to SBUF right
            # away — one head on ACT, one on DVE — so the PSUM accumulators
            # free within the LAG window and the PE never stalls (a stalled
            # PE re-throttles the HAM clock 2.4->1.2 GHz). The recip/
            # broadcast/normalize chain then runs off the critical path.
            uout = [small_p.tile([D + 1, 512], f32, tag=f"uo{w}", name=f"uo{w}")
                    for w in range(2)]
            nc.scalar.copy(out=uout[0][:], in_=outT[0][:])
            nc.vector.tensor_copy(out=uout[1][:], in_=outT[1][:])
            den = small_p.tile([1, 1024], f32, tag="den", name="den")
            for w in range(2):
                nc.vector.tensor_copy(out=den[0:1, w * 512:(w + 1) * 512],
                                      in_=uout[w][D:D + 1, :])
            rec = small_p.tile([1, 1024], f32, tag="rec", name="rec")
            nc.vector.reciprocal_approx_fast(out=rec[0:1, :], in_=den[0:1, :])
            dscr = dram_p.tile([1, 1024], f32, name="dscr")
            nc.sync.dma_start(out=dscr[:], in_=rec[0:1, :])
            for w, h in ((0, h0), (1, h1)):
                row = dscr[0:1, w * 512:(w + 1) * 512]
                rrep = small_p.tile([D, 512], f32, tag=f"rrep{w}", name=f"rrep{w}")
                nc.sync.dma_start(
                    out=rrep[:],
                    in_=bass.AP(tensor=row.tensor, offset=row.offset,
                                ap=[[0, D]] + list(row.ap)[1:]))
                nc.vector.tensor_tensor(
                    attn[h][:, n0:n0 + 512],
                    uout[w][0:D, :],
                    rrep[:],
                    mult,
                )

        def merge_nt(nt):
            for co in range(2):
                for t in range(2 * nt, 2 * nt + 2):
                    ps = spool.tile([P, 1024], f32, tag="sc", name="psm")[:, 0:512]
                    for h in range(H):
                        nc.tensor.matmul(
                            ps[:],
                            wm_h[h][:, co * P:(co + 1) * P],
                            attn[h][:, t * 512:(t + 1) * 512],
                            start=(h == 0), stop=(h == H - 1),
                        )
                    ot = ostage.tile([P, 512], f32, tag="ot", name="ot")
                    if t % 2 == 0:
                        nc.scalar.activation(out=ot[:], in_=ps[:],
                                             func=Ident, bias=bm_t[co][:])
                    else:
                        nc.vector.tensor_scalar_add(ot[:], ps[:], bm_t[co])
                    nc.sync.dma_start(out=y_r[co, :, t * 512:(t + 1) * 512],
                                      in_=ot[:])

        proj_one(0, wkt, src_t, M, K_sb, bk_t)
        proj_one(0, wqt, q_t, NN, Q_sb, bq_t)
        for mc in range(MC):
            proj_vt_chunk(mc)
        for nt in range(NT):
            attention_pair(0, nt)
        proj_one(1, wkt, src_t, M, K_sb, bk_t)
        proj_one(1, wqt, q_t, NN, Q_sb, bq_t)
        for nt in range(NT):
            attention_pair(1, nt)
        merge_nt(0)
        merge_nt(1)

    nc.compile()
    return nc


def _get_nc():
    if "nc" not in _STATE:
        _STATE["nc"] = _build()
    return _STATE["nc"]


def kernel(query, source, Wq, bq, Wk, bk, Wv, bv, Wm, bm):
    import ml_dtypes
    from concourse.bass_utils import run_bass_kernel_spmd

    bf16 = ml_dtypes.bfloat16
    query = np.asarray(query, np.float32).astype(bf16)
    source = np.asarray(source, np.float32).astype(bf16)
    wqT = np.ascontiguousarray(np.asarray(Wq, np.float32).T).astype(bf16)
    wkT = np.ascontiguousarray(np.asarray(Wk, np.float32).T).astype(bf16)
    wvT = np.ascontiguousarray(np.asarray(Wv, np.float32).T).astype(bf16)
    wmT = np.ascontiguousarray(np.asarray(Wm, np.float32).T).astype(bf16)
    bq = np.asarray(bq, np.float32)
    bk = np.asarray(bk, np.float32)
    bv = np.asarray(bv, np.float32)
    bm = np.asarray(bm, np.float32)

    nc = _get_nc()

    in_maps = []
    for c in range(N_CORES):
        b, nh = c // 2, c % 2
        in_maps.append({
            "q": np.ascontiguousarray(query[b, :, nh * NN:(nh + 1) * NN]),
            "src": np.ascontiguousarray(source[b]),
            "wqT": wqT, "wkT": wkT, "wvT": wvT, "wmT": wmT,
            "bq": bq, "bk": bk, "bv": bv, "bm": bm,
        })

    trace = os.environ.get("KERNEL_TRACE") == "1"
    res = run_bass_kernel_spmd(
        nc, in_maps, core_ids=list(range(N_CORES)), trace=trace)
    _STATE["last_result"] = res
    if trace and res.exec_time_ns is not None:
        print(f"HW exec time: {res.exec_time_ns} ns")

    out = np.empty((B, C, N), np.float32)
    for c in range(N_CORES):
        b, nh = c // 2, c % 2
        out[b, :, nh * NN:(nh + 1) * NN] = res.results[c]["y"]
    return out


# revision 27
# speedup vs baseline: 1.4573x; 1.0276x over previous
"""MultiHeadAttention kernel for Trainium2, 8 NeuronCores.

Problem shapes (hardcoded): B=4, C=256, N=M=4096, H=4 heads, D=64 head dim.
reference: Q/K/V = 1x1-conv projections, scores = Q^T K / sqrt(D) per head,
softmax over source dim, out = attn @ V^T, merge projection.

Sharding: 8 cores = (batch b, query-half nh). Each core computes the full
output rows for its (b, n-half): K/V projections are recomputed per n-half
(5% redundant FLOPs) which keeps every core's output slice disjoint — the
host just concatenates, no reductions.

Per-core dataflow (bf16 matmul operands, fp32 PSUM accumulation):
  K  = WkT.T @ src            (c_out on partitions, m free)    [PE]
  Q  = WqT.T @ q              (c_out on partitions, n free)    [PE]
  VT = src.T @ WvT            (m on partitions, c_out free)    [PE]
  attention runs on HEAD PAIRS (2ch, 2ch+1): their score matmuls contract
  only D=64, so the pair occupies PE row-groups 0-63 / 64-127 (tile_position
  auto-derived from base_partition) and the two matmuls execute
  CONCURRENTLY in the systolic array — scores PE time ~halves.
  per (pair ch, n-tile of 1024, m-chunk of 128):
    scoresT_h[m,n] = K_h^T Q_h  both heads, row-tiled          [PE]
    probs = exp(scores/8): one head's chunk on ACT (LUT exp), the
    other on DVE via a bf16 Schraudolph bit-trick:
      bits16 = round(A*s + B), A = 128*log2(e)/8, viewed as bf16
    alternating per m-chunk so every softmax row mixes both paths
    (rel err ~7e-3 end to end)                                 [ACT+DVE]
    outT_h(65,1024) += probs_chunk.T @ [VT_h | ones]  -> row 64 is
    the softmax denominator                                    [PE]
  pair drain: denominators batched (2,1024) -> approx-NR recip [DVE],
    partition-broadcast via a DRAM bounce [DMA], attn_h = outT * r [DVE]
  y = WmT.T @ attn + bm       contract heads, K=64 each        [PE]

Engine balance (the previous version was ACT(exp)-bound at ~343us busy):
exp is split ~50/50 ACT/DVE; K/Q/merge PSUM->SBUF bias-copies run on ACT
(activation Copy with per-partition bias AP); VT bias-adds + attn
normalization stay on DVE. PE does ~200us, ACT/DVE ~200us each.

Hardware landmines (kept working around, see git history of this file):
  - gpsimd.partition_broadcast reads the wrong partition for inputs not
    based at partition 0, and heavy gpsimd SBUF traffic locks the
    DVE-shared port;
  - DMA with partition-shifted or partition-step-0 SBUF APs hangs the
    device (DRAM-side broadcast APs are fine);
  - DVE ops are partition-locked (out/in must share the partition base),
    though a plain reciprocal/copy CAN shift base; the custom-DVE
    reciprocal_approx ops cannot;
  - matmul out must stay within one PSUM bank (<=512 fp32 free).
"""

import os

import numpy as np

N_CORES = 8
B, C = 4, 256
N = M = 4096
H, D = 4, 64
NN = N // 2          # query positions per core
P = 128
NT = NN // 512       # n-tiles per core (4)
MC = M // P          # m-chunks (32)
LAG = 6              # outT matmuls trail scores by LAG m-chunks; also
#                      the window that hides the pair-drain DVE backlog

# Schraudolph bf16 exp: bits16 = round(SCHR_A * s + SCHR_B); includes the
# 1/sqrt(D)=0.125 score scaling. B centers the log-error (C_adj ~ 7.5).
SCHR_A = float(128.0 * 0.125 / np.log(2.0))
SCHR_B = float(128.0 * 127.0 - 7.5)

_STATE: dict = {}


def _build():
    from contextlib import ExitStack

    import concourse.bass as bass
    import concourse.mybir as mybir
    import concourse.tile as tile
    from concourse import bacc

    f32 = mybir.dt.float32
    bf16 = mybir.dt.bfloat16
    i16 = mybir.dt.int16
    Exp = mybir.ActivationFunctionType.Exp
    Ident = mybir.ActivationFunctionType.Identity
    add = mybir.AluOpType.add
    mult = mybir.AluOpType.mult

    nc = bacc.Bacc(
        "TRN2",
        target_bir_lowering=False,
        debug=False,
        enable_asserts=False,
        num_devices=N_CORES,
    )

    q_d = nc.dram_tensor("q", (C, NN), bf16, kind="ExternalInput").ap()
    src_d = nc.dram_tensor("src", (C, M), bf16, kind="ExternalInput").ap()
    wqT_d = nc.dram_tensor("wqT", (C, C), bf16, kind="ExternalInput").ap()
    wkT_d = nc.dram_tensor("wkT", (C, C), bf16, kind="ExternalInput").ap()
    wvT_d = nc.dram_tensor("wvT", (C, C), bf16, kind="ExternalInput").ap()
    wmT_d = nc.dram_tensor("wmT", (C, C), bf16, kind="ExternalInput").ap()
    bq_d = nc.dram_tensor("bq", (C,), f32, kind="ExternalInput").ap()
    bk_d = nc.dram_tensor("bk", (C,), f32, kind="ExternalInput").ap()
    bv_d = nc.dram_tensor("bv", (C,), f32, kind="ExternalInput").ap()
    bm_d = nc.dram_tensor("bm", (C,), f32, kind="ExternalInput").ap()
    y_d = nc.dram_tensor("y", (C, NN), f32, kind="ExternalOutput").ap()

    q_r = q_d.rearrange("(a p) n -> a p n", p=P)
    src_r = src_d.rearrange("(a p) n -> a p n", p=P)
    y_r = y_d.rearrange("(a p) n -> a p n", p=P)

    def chunks(w):
        return w.rearrange("(a p) n -> a p n", p=P)

    with tile.TileContext(nc) as tc, ExitStack() as ctx:
        singles = ctx.enter_context(tc.tile_pool(name="singles", bufs=1))
        # PSUM: scores 3 tiles x 2 banks (3-deep rotation so score matmuls
        # never WAR-wait on the exp of the previous chunk) + outT (two live
        # (65,512) head accumulators) 2 x 1 bank = 8 banks.
        spool = ctx.enter_context(tc.tile_pool(name="scores", bufs=3, space="PSUM"))
        opool = ctx.enter_context(tc.tile_pool(name="outps", bufs=2, space="PSUM"))
        probs_p = ctx.enter_context(tc.tile_pool(name="probs", bufs=LAG + 2))
        small_p = ctx.enter_context(tc.tile_pool(name="small", bufs=2))
        dram_p = ctx.enter_context(tc.tile_pool(name="dram", bufs=2, space="DRAM"))
        ostage = ctx.enter_context(tc.tile_pool(name="ostage", bufs=3))

        # ---- weights / biases ----
        wqt, wkt, wvt = [], [], []
        for ci in range(2):
            for lst, d, nm in ((wqt, wqT_d, "wq"), (wkt, wkT_d, "wk"),
                               (wvt, wvT_d, "wv")):
                t = singles.tile([P, C], bf16, tag=f"{nm}{ci}", name=f"{nm}{ci}")
                nc.gpsimd.dma_start(out=t[:], in_=chunks(d)[ci])
                lst.append(t)
        wm_h = []
        for h in range(H):
            t = singles.tile([D, C], bf16, tag=f"wm{h}", name=f"wm{h}")
            nc.gpsimd.dma_start(out=t[:], in_=wmT_d[h * D:(h + 1) * D, :])
            wm_h.append(t)
        bq_t, bk_t, bm_t = [], [], []
        for ci in range(2):
            for lst, d, nm in ((bq_t, bq_d, "bq"), (bk_t, bk_d, "bk"),
                               (bm_t, bm_d, "bm")):
                t = singles.tile([P, 1], f32, tag=f"{nm}{ci}", name=f"b{nm}{ci}")
                nc.gpsimd.dma_start(out=t[:], in_=d.rearrange("(a p) -> a p", p=P)[ci][:, None])
                lst.append(t)
        bv_rep = singles.tile([P, C], f32, tag="bv_rep", name="bv_rep")
        nc.gpsimd.dma_start(
            out=bv_rep[:],
            in_=bass.AP(tensor=bv_d.tensor, offset=bv_d.offset,
                        ap=[[0, P]] + list(bv_d.ap)),
        )

        # ---- persistent activations ----
        Q_sb = [singles.tile([P, NN], bf16, tag=f"Q{ci}", name=f"Q{ci}") for ci in range(2)]
        K_sb = [singles.tile([P, M], bf16, tag=f"K{ci}", name=f"K{ci}") for ci in range(2)]
        vt = singles.tile([P, MC, H, D + 1], bf16, tag="vt", name="vt")
        attn = [singles.tile([D, NN], bf16, tag=f"attn{h}", name=f"attn{h}") for h in range(H)]

        nc.vector.memset(vt[:, :, :, D:D + 1], 1.0)

        # ---- inputs (chunked DMAs so the first matmul starts early) ----
        inp = ctx.enter_context(tc.tile_pool(name="inp", bufs=1))
        src_t = [inp.tile([P, M], bf16, tag=f"srcin{ci}", name=f"srcin{ci}")
                 for ci in range(2)]
        q_t = [inp.tile([P, NN], bf16, tag=f"qin{ci}", name=f"qin{ci}")
               for ci in range(2)]
        for c4 in range(8):       # src first (K then VT proj need it first),
            for ci in range(2):   # column-chunked so t=0 lands quickly,
                eng = nc.sync if ci == 0 else nc.scalar  # two DGE queues
                eng.dma_start(out=src_t[ci][:, c4 * 512:(c4 + 1) * 512],
                              in_=src_r[ci][:, c4 * 512:(c4 + 1) * 512])
        for c4 in range(4):
            for ci in range(2):
                nc.gpsimd.dma_start(out=q_t[ci][:, c4 * 512:(c4 + 1) * 512],
                                    in_=q_r[ci][:, c4 * 512:(c4 + 1) * 512])

        # ---- projections (bf16 inputs, fp32 PSUM accumulate) ----
        def proj_one(co, wt, xin, xlen, dst, bias):
            # c_out partitions, sequence on free dim; the PSUM->SBUF copies
            # with per-partition bias alternate ACT/DVE so neither engine
            # serializes the PE during projection phases. PSUM comes from
            # the scores pool so these matmuls never wait on the attention
            # drain holding the outT accumulators.
            for t in range(xlen // 512):
                ps = spool.tile([P, 1024], f32, tag="sc", name="ps")[:, 0:512]
                for ci in range(2):
                    nc.tensor.matmul(
                        ps[:],
                        wt[ci][:, co * P:(co + 1) * P],
                        xin[ci][:, t * 512:(t + 1) * 512],
                        start=(ci == 0), stop=(ci == 1),
                    )
                dslc = dst[co][:, t * 512:(t + 1) * 512]
                if t % 2 == 0:
                    nc.scalar.activation(out=dslc, in_=ps[:],
                                         func=Ident, bias=bias[co][:])
                else:
                    nc.vector.tensor_scalar_add(dslc, ps[:], bias[co])

        def proj_vt_chunk(mc):
            # VT = src.T @ WvT (m partitions, c_out free) + bv, stored as
            # per-head [VT_h | ones] blocks of width D+1
            ps = spool.tile([P, 1024], f32, tag="sc", name="psv")[:, 0:C]
            for ci in range(2):
                nc.tensor.matmul(
                    ps[:],
                    src_t[ci][:, mc * P:(mc + 1) * P],
                    wvt[ci][:],
                    start=(ci == 0), stop=(ci == 1),
                )
            nc.vector.tensor_tensor(
                vt[:, mc, :, 0:D],
                ps.rearrange("p (h d) -> p h d", h=H),
                bv_rep.rearrange("p (h d) -> p h d", h=H),
                add,
            )

        # ---- attention on head pairs, n-tiles of 512 ----
        exp_slot = [0]

        def attention_pair(ch, nt):
            h0, h1 = 2 * ch, 2 * ch + 1
            n0 = nt * 512
            outT = [opool.tile([D + 1, 512], f32, tag="outT", name=f"oT{w}")
                    for w in range(2)]
            prs = {}
            for mc in range(MC + LAG):
                if mc < MC:
                    # MIXED score tile: [h0 | h1] halves. The two matmuls
                    # writing it share the same WAR dependency (the tile's
                    # previous exp) so the Tile scheduler keeps them
                    # adjacent — and on HW they run CONCURRENTLY
                    # (row-groups 0-63/64-127 via auto tile_position,
                    # different PSUM banks).
                    sc = spool.tile([P, 1024], f32, tag="sc", name="sc")
                    for w, off in ((0, 0), (1, D)):
                        nc.tensor.matmul(
                            sc[:, w * 512:(w + 1) * 512],
                            K_sb[ch][off:off + D, mc * P:(mc + 1) * P],
                            Q_sb[ch][off:off + D, n0:n0 + 512],
                            start=True, stop=True,
                        )
                    pr = probs_p.tile([P, 1024], bf16, tag="pr", name="pr")
                    # ~41% of chunks on DVE (13 of 32), the rest on ACT:
                    # balances engine time (DVE also carries the normalize/
                    # recip fixed work); the pattern spreads both exp paths
                    # across every softmax row's m-chunks
                    s = exp_slot[0]
                    exp_slot[0] += 1
                    if (s * 13) % 32 < 13:
                        nc.vector.tensor_scalar(
                            pr[:].bitcast(i16), sc[:],
                            SCHR_A, SCHR_B, mult, add)
                    else:
                        nc.scalar.activation(
                            out=pr[:], in_=sc[:], func=Exp, scale=0.125)
                    prs[mc] = pr
                if mc >= LAG:
                    j = mc - LAG
                    pr_j = prs.pop(j)
                    for w, h in ((0, h0), (1, h1)):
                        nc.tensor.matmul(
                            outT[w][:],
                            vt[:, j, h, :],
                            pr_j[:, w * 512:(w + 1) * 512],
                            start=(j == 0), stop=(j == MC - 1),
                        )
            # drain: copy raw outT (incl. denominator row D) to SBUF right
            # away — one head on ACT, one on DVE — so the PSUM accumulators
            # free within the LAG window and the PE never stalls (a stalled
            # PE re-throttles the HAM clock 2.4->1.2 GHz). The recip/
            # broadcast/normalize chain then runs off the critical path.
            uout = [small_p.tile([D + 1, 512], f32, tag=f"uo{w}", name=f"uo{w}")
                    for w in range(2)]
            nc.scalar.copy(out=uout[0][:], in_=outT[0][:])
            nc.scalar.copy(out=uout[1][:], in_=outT[1][:])
            den = small_p.tile([1, 1024], f32, tag="den", name="den")
            for w in range(2):
                nc.vector.tensor_copy(out=den[0:1, w * 512:(w + 1) * 512],
                                      in_=uout[w][D:D + 1, :])
            rec = small_p.tile([1, 1024], f32, tag="rec", name="rec")
            nc.vector.reciprocal_approx_fast(out=rec[0:1, :], in_=den[0:1, :])
            dscr = dram_p.tile([1, 1024], f32, name="dscr")
            nc.sync.dma_start(out=dscr[:], in_=rec[0:1, :])
            for w, h in ((0, h0), (1, h1)):
                row = dscr[0:1, w * 512:(w + 1) * 512]
                rrep = small_p.tile([D, 512], f32, tag=f"rrep{w}", name=f"rrep{w}")
                nc.sync.dma_start(
                    out=rrep[:],
                    in_=bass.AP(tensor=row.tensor, offset=row.offset,
                                ap=[[0, D]] + list(row.ap)[1:]))
                nc.vector.tensor_tensor(
                    attn[h][:, n0:n0 + 512],
                    uout[w][0:D, :],
                    rrep[:],
                    mult,
                )

        def merge_nt(nt):
            for co in range(2):
                for t in range(2 * nt, 2 * nt + 2):
                    ps = spool.tile([P, 1024], f32, tag="sc", name="psm")[:, 0:512]
                    for h in range(H):
                        nc.tensor.matmul(
                            ps[:],
                            wm_h[h][:, co * P:(co + 1) * P],
                            attn[h][:, t * 512:(t + 1) * 512],
                            start=(h == 0), stop=(h == H - 1),
                        )
                    ot = ostage.tile([P, 512], f32, tag="ot", name="ot")
                    if t % 2 == 0:
                        nc.scalar.activation(out=ot[:], in_=ps[:],
                                             func=Ident, bias=bm_t[co][:])
                    else:
                        nc.vector.tensor_scalar_add(ot[:], ps[:], bm_t[co])
                    nc.sync.dma_start(out=y_r[co, :, t * 512:(t + 1) * 512],
                                      in_=ot[:])

        proj_one(0, wkt, src_t, M, K_sb, bk_t)
        proj_one(0, wqt, q_t, NN, Q_sb, bq_t)
        for mc in range(MC):
            proj_vt_chunk(mc)
        for nt in range(NT):
            attention_pair(0, nt)
        proj_one(1, wkt, src_t, M, K_sb, bk_t)
        proj_one(1, wqt, q_t, NN, Q_sb, bq_t)
        for nt in range(NT):
            attention_pair(1, nt)
        merge_nt(0)
        merge_nt(1)

    nc.compile()
    return nc


def _get_nc():
    if "nc" not in _STATE:
        _STATE["nc"] = _build()
    return _STATE["nc"]


def kernel(query, source, Wq, bq, Wk, bk, Wv, bv, Wm, bm):
    import ml_dtypes
    from concourse.bass_utils import run_bass_kernel_spmd

    bf16 = ml_dtypes.bfloat16
    query = np.asarray(query, np.float32).astype(bf16)
    source = np.asarray(source, np.float32).astype(bf16)
    wqT = np.ascontiguousarray(np.asarray(Wq, np.float32).T).astype(bf16)
    wkT = np.ascontiguousarray(np.asarray(Wk, np.float32).T).astype(bf16)
    wvT = np.ascontiguousarray(np.asarray(Wv, np.float32).T).astype(bf16)
    wmT = np.ascontiguousarray(np.asarray(Wm, np.float32).T).astype(bf16)
    bq = np.asarray(bq, np.float32)
    bk = np.asarray(bk, np.float32)
    bv = np.asarray(bv, np.float32)
    bm = np.asarray(bm, np.float32)

    nc = _get_nc()

    in_maps = []
    for c in range(N_CORES):
        b, nh = c // 2, c % 2
        in_maps.append({
            "q": np.ascontiguousarray(query[b, :, nh * NN:(nh + 1) * NN]),
            "src": np.ascontiguousarray(source[b]),
            "wqT": wqT, "wkT": wkT, "wvT": wvT, "wmT": wmT,
            "bq": bq, "bk": bk, "bv": bv, "bm": bm,
        })

    trace = os.environ.get("KERNEL_TRACE") == "1"
    res = run_bass_kernel_spmd(
        nc, in_maps, core_ids=list(range(N_CORES)), trace=trace)
    _STATE["last_result"] = res
    if trace and res.exec_time_ns is not None:
        print(f"HW exec time: {res.exec_time_ns} ns")

    out = np.empty((B, C, N), np.float32)
    for c in range(N_CORES):
        b, nh = c // 2, c % 2
        out[b, :, nh * NN:(nh + 1) * NN] = res.results[c]["y"]
    return out
